# revision 10
# baseline (speedup 1.0000x reference)
"""Band (local) attention kernel for Trainium2, sharded over 8 NeuronCores.

Sharding: sequence-parallel over S (256 queries/core) with a W=64 halo of
keys/values at shard boundaries; projection weights replicated. Host-side
prep: shard slicing, transposes (x^T, W^T), bf16 casts, decay-bias tables.
Device: QKV projections, banded softmax(QK^T)V, output projection, and the
stored attention-weight slices. Host-side post: concat shards, scatter the
boundary attention-weight rows into the (mostly zero) [B,H,S,2W] output.
"""

import sys

if "/opt/trn_rl_repo" not in sys.path:
    sys.path.insert(0, "/opt/trn_rl_repo")

import numpy as np
import ml_dtypes

import concourse.bass as bass
import concourse.tile as tile
from concourse import mybir
from concourse.bass_utils import run_bass_kernel_spmd
from concourse.masks import make_identity

S, D, H, W = 2048, 512, 8, 64
DK = D // H            # 64
NCORES = 8
SL = S // NCORES       # 256 queries per core
KL = SL + W            # 320 valid key rows per core
KP = 384               # key rows padded (zero halo tail)
WIN = 192              # per-query-tile key window (128 + W)
NEG = -1e9

BF = mybir.dt.bfloat16
F32 = mybir.dt.float32
Exp = mybir.ActivationFunctionType.Exp
X = mybir.AxisListType.X


def _build():
    nc = bass.Bass(use_seq_codegen=True)
    xq_t = nc.dram_tensor("xq_t", [D, SL], BF, kind="ExternalInput")
    xk_t = nc.dram_tensor("xk_t", [D, KP], BF, kind="ExternalInput")
    xv_t = nc.dram_tensor("xv_t", [D, KP], BF, kind="ExternalInput")
    wq = nc.dram_tensor("wq", [D, D], BF, kind="ExternalInput")   # Wq.T / 8
    wk = nc.dram_tensor("wk", [D, D], BF, kind="ExternalInput")   # Wk.T
    wv = nc.dram_tensor("wv", [D, D], BF, kind="ExternalInput")   # Wv.T
    wo = nc.dram_tensor("wo", [D, D], BF, kind="ExternalInput")   # Wo.T
    bo = nc.dram_tensor("bo", [D], F32, kind="ExternalInput")
    bias = nc.dram_tensor("bias", [2, 128, WIN], F32, kind="ExternalInput")
    out = nc.dram_tensor("out", [SL, D], F32, kind="ExternalOutput")
    a_top = nc.dram_tensor("a_top", [H, W, 2 * W], F32, kind="ExternalOutput")
    a_bot = nc.dram_tensor("a_bot", [H, W, W + 1], F32, kind="ExternalOutput")

    with tile.TileContext(nc) as tc, \
         tc.tile_pool(name="const", bufs=1) as const, \
         tc.tile_pool(name="persist", bufs=1) as persist, \
         tc.tile_pool(name="work", bufs=4) as work, \
         tc.tile_pool(name="stats", bufs=6) as stats, \
         tc.tile_pool(name="psA", bufs=2, space="PSUM") as psA, \
         tc.tile_pool(name="psS", bufs=2, space="PSUM") as psS, \
         tc.tile_pool(name="psT", bufs=2, space="PSUM") as psT, \
         tc.tile_pool(name="psV", bufs=2, space="PSUM") as psV, \
         tc.tile_pool(name="dstage", bufs=8, space="DRAM") as dstage:

        # ---- loads (each tile: exactly one DMA, zero waits) ----
        def load_wt(dram):
            t = const.tile([128, 4, D], BF, tag=f"w_{dram.name}")
            nc.sync.dma_start(out=t, in_=dram[:].rearrange("(c p) n -> p c n", p=128))
            return t

        wq_sb, wk_sb, wv_sb, wo_sb = (load_wt(d) for d in (wq, wk, wv, wo))

        def load_xt(dram, n):
            t = const.tile([128, 4, n], BF, tag=f"x_{dram.name}")
            nc.scalar.dma_start(out=t, in_=dram[:].rearrange("(c p) s -> p c s", p=128))
            return t

        xqT = load_xt(xq_t, SL)
        xkT = load_xt(xk_t, KP)
        xvT = load_xt(xv_t, KP)

        bias_sb = const.tile([128, 2, WIN], F32, tag="bias")
        nc.gpsimd.dma_start(out=bias_sb, in_=bias[:].rearrange("t q j -> q t j"))

        bo_sb = const.tile([128, D], F32, tag="bo")
        nc.gpsimd.dma_start(
            out=bo_sb,
            in_=bass.AP(tensor=bo[:].tensor, offset=0, ap=[[0, 128], [1, D]]),
        )

        ident = const.tile([128, 128], BF, tag="ident")
        make_identity(nc, ident)

        # ---- projections ----
        # Q^T/K^T in [head-dim, seq] layout, two heads per 128-partition group.
        QT = persist.tile([128, 4, SL], BF, tag="QT")
        KT = persist.tile([128, 4, KL], BF, tag="KT")
        for g in range(4):
            ps = psA.tile([128, SL], F32, tag="psA")
            for ci in range(4):
                nc.tensor.matmul(ps, wq_sb[:, ci, 128 * g:128 * (g + 1)],
                                 xqT[:, ci, :], start=(ci == 0), stop=(ci == 3))
            nc.any.tensor_copy(QT[:, g, :], ps)
            ps2 = psA.tile([128, KL], F32, tag="psA")
            for ci in range(4):
                nc.tensor.matmul(ps2, wk_sb[:, ci, 128 * g:128 * (g + 1)],
                                 xkT[:, ci, :KL], start=(ci == 0), stop=(ci == 3))
            nc.any.tensor_copy(KT[:, g, :], ps2)

        # V in natural [key-row, all-head] layout, 3 row tiles (128/128/64).
        V = persist.tile([128, 3, D], BF, tag="V")
        for vt in range(3):
            nr = 128 if vt < 2 else 64
            ps = psA.tile([128, D], F32, tag="psA")
            for ci in range(4):
                nc.tensor.matmul(ps[:nr, :], xvT[:, ci, 128 * vt:128 * vt + nr],
                                 wv_sb[:, ci, :], start=(ci == 0), stop=(ci == 3))
            nc.any.tensor_copy(V[:nr, vt, :], ps[:nr, :])

        # ---- banded attention ----
        # phase A: scores + softmax for all (head, qtile); probs persisted
        PB = persist.tile([128, 16, WIN], BF, tag="PB")
        it = 0
        for g in range(4):
            for hh in range(2):
                h = 2 * g + hh
                hp = slice(64 * hh, 64 * hh + 64)
                for t in range(2):
                    j0 = 128 * t
                    ps_s = psS.tile([128, KL], F32, tag="psS")
                    nc.tensor.matmul(ps_s, QT[hp, g, j0:j0 + 128], KT[hp, g, :],
                                     start=True, stop=True)
                    sc = work.tile([128, WIN], F32, tag="sc")
                    nc.vector.tensor_add(sc, ps_s[:, j0:j0 + WIN], bias_sb[:, t, :])
                    nm = stats.tile([128, 1], F32, tag="nm")
                    nc.vector.reduce_max(nm, sc, axis=X, negate=True)
                    e = work.tile([128, WIN], F32, tag="e")
                    ssum = stats.tile([128, 1], F32, tag="ssum")
                    nc.scalar.activation(e, sc, Exp, bias=nm, scale=1.0,
                                         accum_out=ssum)
                    r = stats.tile([128, 1], F32, tag="r")
                    nc.vector.reciprocal(r, ssum)
                    nc.vector.tensor_scalar_mul(PB[:, it, :], e, r)

                    if t == 0:
                        at = work.tile([64, 2 * W], F32, tag="atop")
                        nc.scalar.activation(at, e[0:64, 64:192],
                                             mybir.ActivationFunctionType.Copy,
                                             scale=r[0:64])
                        nc.scalar.dma_start(out=a_top[h], in_=at)
                    else:
                        pstg = work.tile([64, WIN], F32, tag="pstg")
                        nc.scalar.activation(pstg, e[64:128, :],
                                             mybir.ActivationFunctionType.Copy,
                                             scale=r[64:128])
                        flat = dstage.tile([64 * 193 + 64], F32, tag="dstage")
                        wview = flat[0:64 * WIN].rearrange("(a b) -> a b", b=WIN)
                        nc.scalar.dma_start(out=wview, in_=pstg)
                        rview = flat[0:64 * 193].rearrange("(a b) -> a b", b=193)
                        nc.gpsimd.dma_start(out=a_bot[h],
                                            in_=rview[:, 64:64 + W + 1])
                    it += 1

        # phase B: probs^T and P.V, dense on PE
        outT = persist.tile([128, 4, SL], BF, tag="outT")
        it = 0
        for g in range(4):
            for hh in range(2):
                h = 2 * g + hh
                hp = slice(64 * hh, 64 * hh + 64)
                for t in range(2):
                    j0 = 128 * t
                    tp0 = psT.tile([128, 128], BF, tag="psT")
                    nc.tensor.transpose(tp0, PB[:, it, 0:128], ident)
                    tp1 = psT.tile([128, 128], BF, tag="psT")
                    nc.tensor.transpose(tp1[0:64, :], PB[:, it, 128:192], ident)
                    pT0 = work.tile([128, 128], BF, tag="pT0")
                    nc.any.tensor_copy(pT0, tp0)
                    pT1 = work.tile([64, 128], BF, tag="pT1")
                    nc.any.tensor_copy(pT1, tp1[0:64, :])

                    po = psV.tile([64, 128], F32, tag="psV")
                    nc.tensor.matmul(po, V[:, t, 64 * h:64 * h + 64], pT0,
                                     start=True, stop=False)
                    nc.tensor.matmul(po, V[0:64, t + 1, 64 * h:64 * h + 64], pT1,
                                     start=False, stop=True)
                    nc.any.tensor_copy(outT[hp, g, j0:j0 + 128], po)
                    it += 1

        # ---- output projection ----
        for qt in range(2):
            ps = psA.tile([128, D], F32, tag="psA")
            for g in range(4):
                nc.tensor.matmul(ps, outT[:, g, 128 * qt:128 * (qt + 1)],
                                 wo_sb[:, g, :], start=(g == 0), stop=(g == 3))
            ob = work.tile([128, D], F32, tag="ob")
            nc.vector.tensor_add(ob, ps, bo_sb)
            nc.scalar.dma_start(out=out[128 * qt:128 * (qt + 1), :], in_=ob)

    _split_multi_waits(nc)
    return nc


def _split_multi_waits(nc):
    """This toolchain's walrus accepts at most one sem-wait per regular
    instruction: move extra waits onto same-engine NoOps placed just before."""
    skip = {"InstEventSemaphore", "InstUnconditionalBranch",
            "InstConditionalBranch", "InstISA"}
    n = 0
    for f in nc.m.functions:
        for b in f.blocks:
            new = []
            for ins in b.instructions:
                si = ins.sync_info
                ws = list(si.on_wait) if (si and si.on_wait) else []
                if len(ws) > 1 and type(ins).__name__ not in skip:
                    for w in ws[:-1]:
                        n += 1
                        nop = mybir.InstNoOp(
                            name=f"waitsplit-{n}", ins=[], outs=[],
                            sync_info=mybir.SyncInfo(on_wait=[w], on_update=[]),
                        )
                        nop.engine = ins.engine
                        new.append(nop)
                    ins.sync_info = mybir.SyncInfo(
                        on_wait=[ws[-1]], on_update=list(si.on_update or []))
                new.append(ins)
            b.instructions[:] = new


_NC_CACHE = None
LAST_RESULT = None


def _get_nc():
    global _NC_CACHE
    if _NC_CACHE is None:
        _NC_CACHE = _build()
    return _NC_CACHE


def _host_bias(core):
    d = np.arange(WIN)[None, :] - np.arange(128)[:, None]
    band = (d >= 0) & (d <= W)
    B = np.where(band, 0.1 * np.exp(-0.1 * (W - d)), NEG).astype(np.float32)
    if core == 0:
        B0 = B.copy()
        B0[:, :W] = NEG           # keys < 0 don't exist
        return np.stack([B0, B])
    return np.stack([B, B])


def kernel(query, key, value, Wq, Wk, Wv, Wo, bo):
    bf = ml_dtypes.bfloat16
    q = np.asarray(query, np.float32).reshape(S, D)
    k = np.asarray(key, np.float32).reshape(S, D)
    v = np.asarray(value, np.float32).reshape(S, D)
    Wq, Wk, Wv, Wo = (np.asarray(a, np.float32) for a in (Wq, Wk, Wv, Wo))
    bo = np.ascontiguousarray(np.asarray(bo, np.float32))

    wq_t = np.ascontiguousarray(Wq.T / np.sqrt(DK)).astype(bf)
    wk_t = np.ascontiguousarray(Wk.T).astype(bf)
    wv_t = np.ascontiguousarray(Wv.T).astype(bf)
    wo_t = np.ascontiguousarray(Wo.T).astype(bf)

    qT = np.ascontiguousarray(q.T).astype(bf)           # [D, S]
    kg = np.zeros((D, W + S + (KP - KL)), np.float32)   # [D, 2176] halo-padded
    vg = np.zeros_like(kg)
    kg[:, W:W + S] = k.T
    vg[:, W:W + S] = v.T
    kg = kg.astype(bf)
    vg = vg.astype(bf)

    in_maps = []
    for c in range(NCORES):
        in_maps.append({
            "xq_t": np.ascontiguousarray(qT[:, c * SL:(c + 1) * SL]),
            "xk_t": np.ascontiguousarray(kg[:, c * SL:c * SL + KP]),
            "xv_t": np.ascontiguousarray(vg[:, c * SL:c * SL + KP]),
            "wq": wq_t, "wk": wk_t, "wv": wv_t, "wo": wo_t,
            "bo": bo, "bias": _host_bias(c),
        })

    res = run_bass_kernel_spmd(_get_nc(), in_maps, list(range(NCORES)))
    global LAST_RESULT
    LAST_RESULT = res
    rs = res.results

    output = np.concatenate([r["out"] for r in rs], axis=0).reshape(1, S, D)
    attn = np.zeros((1, H, S, 2 * W), np.float32)
    attn[0, :, :W, :] = rs[0]["a_top"]
    attn[0, :, S - W:, :W + 1] = rs[7]["a_bot"]
    return output, attn
